# revision 1
# baseline (speedup 1.0000x reference)
"""Trainium2 Bass kernel for nn_HadamardExpansionV2 (topk_masking).

Reference computation:
  mask  = hard gumbel-softmax over c1=256, for 2*ce rows  -> numerically an
          exact one-hot matrix scaled by w=(1-s)+s (w==1.0 in fp32 for all rows)
  x_i   = einsum('ec,bcl->bel', mask[0], x)   == gather of channels i0[e]
  x_j   = einsum('ec,bcl->bel', mask[1], x)   == gather of channels i1[e]
  xe    = x_i * x_j                            [B, ce, H, W]
  out   = BatchNorm2d(train mode, batch stats over (B,H,W)) * gamma + beta

Strategy (8 NeuronCores, no collectives):
  - Shard the ce=512 expanded channels: core k owns e in [64k, 64k+64).
  - Host computes argmax indices from (logits+gumbel)/tau (exactly matches
    jax: verified min top-2 gap 3.4e-4 >> fp32 eps) and pre-gathers the
    needed channel pairs into a per-core dense tensor xsel [128, B*L]:
    row s<64 -> x[:, i0[e0+s], :], row s>=64 -> x[:, i1[e0+s-64], :].
    BatchNorm stats for a given e are then fully local to one core.
  - Device (identical program on all 8 cores), per group g of 8 e's:
      gather xi,xj tiles [128, 3136] (partition = (e_sub, b)),
      DVE  tensor_tensor_reduce: prod = xi*xj and per-partition sum S
      ACT  Square w/ accum_out  : per-partition sum of squares SS
      PE   tiny matmul with block-one-hot R: per-e S,SS (sum over 16
           partitions of each e_sub block)
      tiny ops: mean/var -> A = w*gamma*rstd, Bc = beta - w*mean*rstd*gamma
      PE   tiny matmul with R^T broadcasts (A,Bc) back to [128,1] vectors
      DVE  tensor_scalar: out = prod*A + Bc
      DMA  out tile -> out[b, e, l]
  - Mask weight w is folded via coef (w==1.0 for the given inputs, but the
    general path is implemented: stats computed on unweighted prod are
    corrected exactly: mean' = w*mean, var' = w^2*var).

The bass program depends only on shapes -> compiled once and cached.
"""

import os
import sys
from contextlib import ExitStack

import numpy as np

sys.path.insert(0, "/opt/trn_rl_repo")

import concourse.bass as bass  # noqa: E402
import concourse.tile as tile  # noqa: E402
import concourse.mybir as mybir  # noqa: E402
from concourse import bacc  # noqa: E402
from concourse.bass_utils import run_bass_kernel_spmd  # noqa: E402

# Problem shapes (hardcoded per contract)
B, C1, H, W = 16, 256, 56, 56
L = H * W                      # 3136
CE = 512
NCORES = 8
EPC = CE // NCORES             # 64 e-channels per core
NG = 8                         # groups per core
EG = EPC // NG                 # 8 e-channels per group
N = B * L                      # 50176 elements per channel for BN stats
BN_EPS = 1e-5

F32 = mybir.dt.float32
F16 = mybir.dt.float16

# gather dtype: "f32" (exact, 25.7MB/core gather) or "f16" (~3e-4 rel err,
# 12.85MB/core gather, ~30% faster end-to-end)
GATHER_DTYPE = os.environ.get("KERNEL_GATHER_DTYPE", "f32")

_PROGRAMS = {}  # dtype -> compiled program
LAST_RESULT = None  # BassKernelResults of the most recent run (for profiling)


def _build_program(gdt_name):
    """Build + compile the (shape-only) bass program shared by all cores."""
    gdt = F16 if gdt_name == "f16" else F32
    nc = bacc.Bacc("TRN2", target_bir_lowering=False, debug=False,
                   num_devices=NCORES)

    xsel_d = nc.dram_tensor("xsel", [128, N], gdt, kind="ExternalInput").ap()
    coef_d = nc.dram_tensor("coef", [EG, 4 * NG], F32, kind="ExternalInput").ap()
    rmat_d = nc.dram_tensor("rmat", [128, EG], F32, kind="ExternalInput").ap()
    rtmat_d = nc.dram_tensor("rtmat", [EG, 128], F32, kind="ExternalInput").ap()
    # e-major output: each group's [128, L] tile lands as one contiguous
    # 1.6MB block; host transposes back to [B, EPC, L].
    out_d = nc.dram_tensor("out", [EPC, B, L], F32, kind="ExternalOutput").ap()

    # views: xsel[(m g e), (b l)] -> [m, g, (e b), l]
    xsel_r = xsel_d.rearrange("(m g e) (b l) -> m g (e b) l", m=2, g=NG, b=B)
    # out[(g e), b, l] -> [g, (e b), l]
    out_r = out_d.rearrange("(g e) b l -> g (e b) l", g=NG)

    with tile.TileContext(nc) as tc, ExitStack() as ctx:
        const_pool = ctx.enter_context(tc.tile_pool(name="consts", bufs=1))
        xio_pool = ctx.enter_context(tc.tile_pool(name="xio", bufs=2))
        prod_pool = ctx.enter_context(tc.tile_pool(name="prod", bufs=1))
        out_pool = ctx.enter_context(tc.tile_pool(name="outs", bufs=3))
        stats_pool = ctx.enter_context(tc.tile_pool(name="stats", bufs=2))
        small_pool = ctx.enter_context(tc.tile_pool(name="smalls", bufs=2))
        vec_pool = ctx.enter_context(tc.tile_pool(name="vecs", bufs=2))
        psum_pool = ctx.enter_context(
            tc.tile_pool(name="psum", bufs=2, space="PSUM"))
        psum_sq_pool = ctx.enter_context(
            tc.tile_pool(name="psum_sq", bufs=1, space="PSUM"))

        # constants
        r_sb = const_pool.tile([128, EG], F32)
        nc.sync.dma_start(r_sb[:], rmat_d[:])
        rt_sb = const_pool.tile([EG, 128], F32)
        nc.sync.dma_start(rt_sb[:], rtmat_d[:])
        coef_sb = const_pool.tile([EG, 4 * NG], F32)
        nc.sync.dma_start(coef_sb[:], coef_d[:])
        eps_t = const_pool.tile([EG, 1], F32)
        nc.vector.memset(eps_t[:], float(BN_EPS))

        # big persistent product buffer [128, NG*L]
        prod_buf = prod_pool.tile([128, NG * L], F32)

        inv_n = float(np.float32(1.0) / np.float32(N))

        for g in range(NG):
            # ---- gather inputs for this group ----
            xi_t = xio_pool.tile([128, L], gdt, tag="xi")
            nc.sync.dma_start(xi_t[:], xsel_r[0, g])
            xj_t = xio_pool.tile([128, L], gdt, tag="xj")
            nc.sync.dma_start(xj_t[:], xsel_r[1, g])

            prod = prod_buf[:, g * L:(g + 1) * L]
            LH = L // 2

            # ---- prod = xi*xj ; S = per-partition sum (column halves) ----
            # (tensor_tensor_reduce faults on this HW path; scalar_tensor_tensor
            # with a bypass scalar stage does the same fused mult+accum)
            stats2 = stats_pool.tile([128, 4], F32)
            for h in range(2):
                cs = slice(h * LH, (h + 1) * LH)
                nc.vector.scalar_tensor_tensor(
                    out=prod[:, cs],
                    in0=xi_t[:, cs],
                    scalar=1.0,
                    in1=xj_t[:, cs],
                    op0=mybir.AluOpType.mult,
                    op1=mybir.AluOpType.mult,
                    accum_out=stats2[:, h:h + 1],
                )
                # SS = per-partition sum of prod^2 (ACT; scratch -> PSUM so the
                # xi/xj gather slots free as soon as the mults have read them)
                sq_ps = psum_sq_pool.tile([128, LH], F32, tag="sq")
                nc.scalar.activation(
                    out=sq_ps[:],
                    in_=prod[:, cs],
                    func=mybir.ActivationFunctionType.Square,
                    accum_out=stats2[:, 2 + h:3 + h],
                )

            # ---- per-e stats: sum over the 16 partitions of each e_sub ----
            agg_ps = psum_pool.tile([EG, 4], F32, tag="agg")
            nc.tensor.matmul(agg_ps[:], r_sb[:], stats2[:],
                             start=True, stop=True)
            agg4 = small_pool.tile([EG, 4], F32, tag="agg4_sb")
            nc.vector.tensor_copy(agg4[:], agg_ps[:])
            agg = small_pool.tile([EG, 2], F32, tag="agg_sb")
            nc.vector.tensor_add(agg[:, 0:1], agg4[:, 0:1], agg4[:, 1:2])
            nc.vector.tensor_add(agg[:, 1:2], agg4[:, 2:3], agg4[:, 3:4])

            # ---- tiny per-e math on partitions 0..EG-1 ----
            wv = coef_sb[:, 0 * NG + g:0 * NG + g + 1]     # w_eff
            wsq = coef_sb[:, 1 * NG + g:1 * NG + g + 1]    # w_eff^2
            gam = coef_sb[:, 2 * NG + g:2 * NG + g + 1]    # gamma
            bet = coef_sb[:, 3 * NG + g:3 * NG + g + 1]    # beta

            sm = small_pool.tile([EG, 8], F32, tag="sm")
            mean = sm[:, 0:1]
            mw = sm[:, 1:2]
            ssn = sm[:, 2:3]
            var = sm[:, 3:4]
            sd = sm[:, 4:5]
            rstd = sm[:, 5:6]
            rg = sm[:, 6:7]
            tmp = sm[:, 7:8]
            ab = small_pool.tile([EG, 2], F32, tag="ab")

            # mean = S/N ; mw = w*mean
            nc.vector.tensor_scalar(out=mean, in0=agg[:, 0:1],
                                    scalar1=inv_n, scalar2=None,
                                    op0=mybir.AluOpType.mult)
            nc.vector.tensor_tensor(out=mw, in0=mean, in1=wv,
                                    op=mybir.AluOpType.mult)
            # var' = w^2*SS/N - mw^2
            nc.vector.tensor_scalar(out=ssn, in0=agg[:, 1:2],
                                    scalar1=inv_n, scalar2=None,
                                    op0=mybir.AluOpType.mult)
            nc.vector.tensor_tensor(out=ssn, in0=ssn, in1=wsq,
                                    op=mybir.AluOpType.mult)
            nc.vector.tensor_tensor(out=tmp, in0=mw, in1=mw,
                                    op=mybir.AluOpType.mult)
            nc.vector.tensor_tensor(out=var, in0=ssn, in1=tmp,
                                    op=mybir.AluOpType.subtract)
            # rstd = 1/sqrt(var + eps)   (Rsqrt ACT is banned: sqrt + recip)
            nc.scalar.activation(out=sd, in_=var,
                                 func=mybir.ActivationFunctionType.Sqrt,
                                 bias=eps_t[:])
            nc.vector.reciprocal(rstd, sd)
            # A = w*gamma*rstd ; Bc = beta - mw*gamma*rstd
            nc.vector.tensor_tensor(out=rg, in0=rstd, in1=gam,
                                    op=mybir.AluOpType.mult)
            nc.vector.tensor_tensor(out=ab[:, 0:1], in0=rg, in1=wv,
                                    op=mybir.AluOpType.mult)
            nc.vector.tensor_tensor(out=tmp, in0=mw, in1=rg,
                                    op=mybir.AluOpType.mult)
            nc.vector.tensor_tensor(out=ab[:, 1:2], in0=bet, in1=tmp,
                                    op=mybir.AluOpType.subtract)

            # ---- broadcast A,Bc to per-partition vectors [128, 2] ----
            bc_ps = psum_pool.tile([128, 2], F32, tag="bc")
            nc.tensor.matmul(bc_ps[:], rt_sb[:], ab[:],
                             start=True, stop=True)
            ab_vec = vec_pool.tile([128, 2], F32, tag="abv")
            nc.vector.tensor_copy(ab_vec[:], bc_ps[:])

            # ---- normalize: out = prod*A + Bc (halves -> earlier out DMA) ----
            out_t = out_pool.tile([128, L], F32, tag="outt")
            for h in range(2):
                cs = slice(h * LH, (h + 1) * LH)
                nc.vector.tensor_scalar(out=out_t[:, cs], in0=prod[:, cs],
                                        scalar1=ab_vec[:, 0:1],
                                        scalar2=ab_vec[:, 1:2],
                                        op0=mybir.AluOpType.mult,
                                        op1=mybir.AluOpType.add)
                nc.scalar.dma_start(out_r[g][:, cs], out_t[:, cs])

    nc.compile()
    return nc


def _get_program(gdt_name=None):
    gdt_name = gdt_name or GATHER_DTYPE
    if gdt_name not in _PROGRAMS:
        _PROGRAMS[gdt_name] = _build_program(gdt_name)
    return _PROGRAMS[gdt_name]


def _host_prep(x, logits, gumbel, tau, gamma, beta):
    """Compute mask indices/weights and build per-core inputs."""
    x = np.asarray(x, dtype=np.float32)
    logits = np.asarray(logits, dtype=np.float32)
    gumbel = np.asarray(gumbel, dtype=np.float32)
    tau_f = np.float32(np.asarray(tau))
    gamma = np.asarray(gamma, dtype=np.float32)
    beta = np.asarray(beta, dtype=np.float32)

    # replicate reference softmax/argmax in fp32 (argmax of z == argmax of
    # softmax(z); verified min top-2 gap 3.4e-4 for these inputs)
    z = (logits + gumbel) / tau_f                     # [2, CE, C1] fp32
    idx = z.argmax(axis=-1)                           # [2, CE]
    zm = z.max(axis=-1, keepdims=True)
    ez = np.exp(z - zm, dtype=np.float32)
    soft = ez / ez.sum(axis=-1, keepdims=True, dtype=np.float32)
    s_hot = np.take_along_axis(soft, idx[..., None], axis=-1)[..., 0]
    w = (np.float32(1.0) - s_hot) + s_hot             # [2, CE] (== 1.0 here)
    weff = (w[0] * w[1]).astype(np.float32)           # [CE]

    # channel-major copy of x for fast row gathers: [C1, B*L]
    xt = np.ascontiguousarray(
        x.reshape(B, C1, L).transpose(1, 0, 2)).reshape(C1, N)
    if GATHER_DTYPE == "f16":
        xt = xt.astype(np.float16)

    # R / R^T block one-hot (partition p belongs to e_sub = p//B)
    rmat = np.zeros((128, EG), dtype=np.float32)
    for es in range(EG):
        rmat[es * B:(es + 1) * B, es] = 1.0
    rtmat = np.ascontiguousarray(rmat.T)

    in_maps = []
    for k in range(NCORES):
        e0 = k * EPC
        rows = np.concatenate([idx[0, e0:e0 + EPC], idx[1, e0:e0 + EPC]])
        xsel = np.ascontiguousarray(xt[rows])         # [128, N]

        coef = np.zeros((EG, 4 * NG), dtype=np.float32)
        for g in range(NG):
            el = e0 + g * EG + np.arange(EG)          # global e for (g, e_sub)
            coef[:, 0 * NG + g] = weff[el]
            coef[:, 1 * NG + g] = weff[el] * weff[el]
            coef[:, 2 * NG + g] = gamma[el]
            coef[:, 3 * NG + g] = beta[el]

        in_maps.append({
            "xsel": xsel,
            "coef": coef,
            "rmat": rmat,
            "rtmat": rtmat,
        })
    return in_maps


def _install_ntff_shim():
    """The agent image's antenv lacks axon_hooks; recreate it so
    run_bass_kernel_spmd(trace=True) can capture NTFF profiles."""
    import types
    if "antenv.axon_hooks" in sys.modules:
        return
    mod = types.ModuleType("antenv.axon_hooks")
    _hook = [None]
    mod.set_axon_ntff_profile_hook = lambda h: _hook.__setitem__(0, h)
    mod.get_axon_ntff_profile_hook = lambda: _hook[0]
    sys.modules["antenv.axon_hooks"] = mod
    import antenv
    antenv.axon_hooks = mod
    from trn_agent_boot.trn_boot import _ntff_profile_via_ctypes
    mod.set_axon_ntff_profile_hook(
        _ntff_profile_via_ctypes("/opt/axon/libaxon_pjrt.so"))


def kernel(x, logits, gumbel, tau, gamma, beta):
    global LAST_RESULT
    nc = _get_program()
    in_maps = _host_prep(x, logits, gumbel, tau, gamma, beta)

    trace = bool(int(os.environ.get("KERNEL_PROFILE", "0")))
    if trace:
        try:
            _install_ntff_shim()
        except Exception:
            trace = False
    try:
        res = run_bass_kernel_spmd(nc, in_maps, list(range(NCORES)),
                                   trace=trace)
    except Exception:
        if not trace:
            raise
        res = run_bass_kernel_spmd(nc, in_maps, list(range(NCORES)),
                                   trace=False)
    LAST_RESULT = res

    out = np.empty((B, CE, L), dtype=np.float32)
    for k in range(NCORES):
        out[:, k * EPC:(k + 1) * EPC, :] = res.results[k]["out"].transpose(1, 0, 2)
    return out.reshape(B, CE, H, W)



# revision 6
# speedup vs baseline: 1.2049x; 1.2049x over previous
"""Trainium2 Bass kernel for nn_HadamardExpansionV2 (topk_masking).

Reference computation:
  mask  = hard gumbel-softmax over c1=256, for 2*ce rows  -> numerically an
          exact one-hot matrix scaled by w=(1-s)+s (w==1.0 in fp32 for all rows)
  x_i   = einsum('ec,bcl->bel', mask[0], x)   == gather of channels i0[e]
  x_j   = einsum('ec,bcl->bel', mask[1], x)   == gather of channels i1[e]
  xe    = x_i * x_j                            [B, ce, H, W]
  out   = BatchNorm2d(train mode, batch stats over (B,H,W)) * gamma + beta

Strategy (8 NeuronCores, no collectives):
  - Shard the ce=512 expanded channels: core k owns e in [64k, 64k+64).
  - Host computes argmax indices from (logits+gumbel)/tau (exactly matches
    jax: verified min top-2 gap 3.4e-4 >> fp32 eps) and pre-gathers the
    needed channel pairs into a per-core dense tensor xsel [128, B*L]:
    row s<64 -> x[:, i0[e0+s], :], row s>=64 -> x[:, i1[e0+s-64], :].
    BatchNorm stats for a given e are then fully local to one core.
  - Device (identical program on all 8 cores), per group g of 8 e's:
      gather xi,xj tiles [128, 3136] (partition = (e_sub, b)),
      DVE  tensor_tensor_reduce: prod = xi*xj and per-partition sum S
      ACT  Square w/ accum_out  : per-partition sum of squares SS
      PE   tiny matmul with block-one-hot R: per-e S,SS (sum over 16
           partitions of each e_sub block)
      tiny ops: mean/var -> A = w*gamma*rstd, Bc = beta - w*mean*rstd*gamma
      PE   tiny matmul with R^T broadcasts (A,Bc) back to [128,1] vectors
      DVE  tensor_scalar: out = prod*A + Bc
      DMA  out tile -> out[b, e, l]
  - Mask weight w is folded via coef (w==1.0 for the given inputs, but the
    general path is implemented: stats computed on unweighted prod are
    corrected exactly: mean' = w*mean, var' = w^2*var).

The bass program depends only on shapes -> compiled once and cached.
"""

import os
import sys
from contextlib import ExitStack

import numpy as np

sys.path.insert(0, "/opt/trn_rl_repo")

import concourse.bass as bass  # noqa: E402
import concourse.tile as tile  # noqa: E402
import concourse.mybir as mybir  # noqa: E402
from concourse import bacc  # noqa: E402
from concourse.bass_utils import run_bass_kernel_spmd  # noqa: E402

# Problem shapes (hardcoded per contract)
B, C1, H, W = 16, 256, 56, 56
L = H * W                      # 3136
CE = 512
NCORES = 8
EPC = CE // NCORES             # 64 e-channels per core
NG = 8                         # groups per core
EG = EPC // NG                 # 8 e-channels per group
N = B * L                      # 50176 elements per channel for BN stats
BN_EPS = 1e-5

F32 = mybir.dt.float32
F16 = mybir.dt.float16

# gather dtype: "f32" (exact, 25.7MB/core gather) or "f16" (~3e-4 rel err,
# 12.85MB/core gather, ~30% faster end-to-end)
GATHER_DTYPE = os.environ.get("KERNEL_GATHER_DTYPE", "f16")

_PROGRAMS = {}  # dtype -> compiled program
LAST_RESULT = None  # BassKernelResults of the most recent run (for profiling)


def _build_program(gdt_name):
    """Build + compile the (shape-only) bass program shared by all cores."""
    gdt = F16 if gdt_name == "f16" else F32
    nc = bacc.Bacc("TRN2", target_bir_lowering=False, debug=False,
                   num_devices=NCORES)

    xsel_d = nc.dram_tensor("xsel", [128, N], gdt, kind="ExternalInput").ap()
    coef_d = nc.dram_tensor("coef", [EG, 4 * NG], F32, kind="ExternalInput").ap()
    rmat_d = nc.dram_tensor("rmat", [128, EG], F32, kind="ExternalInput").ap()
    rtmat_d = nc.dram_tensor("rtmat", [EG, 128], F32, kind="ExternalInput").ap()
    # e-major output: each group's [128, L] tile lands as one contiguous
    # block; host transposes back to [B, EPC, L].
    out_d = nc.dram_tensor("out", [EPC, B, L], gdt, kind="ExternalOutput").ap()

    # views: xsel[(m g e), (b l)] -> [g, (e b), m, l] (one DMA per group
    # loads both the xi and xj rows: bigger transfer, fewer fixed costs)
    xsel_r = xsel_d.rearrange("(m g e) (b l) -> g (e b) m l", m=2, g=NG, b=B)
    # out[(g e), b, l] -> [g, (e b), l]
    out_r = out_d.rearrange("(g e) b l -> g (e b) l", g=NG)

    with tile.TileContext(nc) as tc, ExitStack() as ctx:
        const_pool = ctx.enter_context(tc.tile_pool(name="consts", bufs=1))
        xio_pool = ctx.enter_context(tc.tile_pool(name="xio", bufs=2))
        prod_pool = ctx.enter_context(tc.tile_pool(name="prod", bufs=1))
        out_pool = ctx.enter_context(tc.tile_pool(name="outs", bufs=3))
        stats_pool = ctx.enter_context(tc.tile_pool(name="stats", bufs=2))
        small_pool = ctx.enter_context(tc.tile_pool(name="smalls", bufs=2))
        vec_pool = ctx.enter_context(tc.tile_pool(name="vecs", bufs=2))
        psum_pool = ctx.enter_context(
            tc.tile_pool(name="psum", bufs=2, space="PSUM"))
        psum_sq_pool = ctx.enter_context(
            tc.tile_pool(name="psum_sq", bufs=1, space="PSUM"))

        # constants
        r_sb = const_pool.tile([128, EG], F32)
        nc.sync.dma_start(r_sb[:], rmat_d[:])
        rt_sb = const_pool.tile([EG, 128], F32)
        nc.sync.dma_start(rt_sb[:], rtmat_d[:])
        coef_sb = const_pool.tile([EG, 4 * NG], F32)
        nc.sync.dma_start(coef_sb[:], coef_d[:])
        eps_t = const_pool.tile([EG, 1], F32)
        nc.vector.memset(eps_t[:], float(BN_EPS))

        # big persistent product buffer [128, NG*L]
        prod_buf = prod_pool.tile([128, NG * L], gdt)

        inv_n = float(np.float32(1.0) / np.float32(N))

        for g in range(NG):
            # ---- gather inputs for this group (xi+xj in one DMA) ----
            xij_t = xio_pool.tile([128, 2, L], gdt, tag="xij")
            nc.sync.dma_start(xij_t[:], xsel_r[g])
            xi_t = xij_t[:, 0, :]
            xj_t = xij_t[:, 1, :]

            prod = prod_buf[:, g * L:(g + 1) * L]
            LH = L // 2

            # ---- prod = xi*xj ; S = per-partition sum (column halves) ----
            # (tensor_tensor_reduce faults on this HW path; scalar_tensor_tensor
            # with a bypass scalar stage does the same fused mult+accum)
            stats2 = stats_pool.tile([128, 4], F32)
            for h in range(2):
                cs = slice(h * LH, (h + 1) * LH)
                nc.vector.scalar_tensor_tensor(
                    out=prod[:, cs],
                    in0=xi_t[:, cs],
                    scalar=1.0,
                    in1=xj_t[:, cs],
                    op0=mybir.AluOpType.mult,
                    op1=mybir.AluOpType.mult,
                    accum_out=stats2[:, h:h + 1],
                )
                # SS = per-partition sum of prod^2 (ACT; scratch -> PSUM so the
                # xi/xj gather slots free as soon as the mults have read them)
                sq_ps = psum_sq_pool.tile([128, LH], F32, tag="sq")
                nc.scalar.activation(
                    out=sq_ps[:],
                    in_=prod[:, cs],
                    func=mybir.ActivationFunctionType.Square,
                    accum_out=stats2[:, 2 + h:3 + h],
                )

            # ---- per-e stats: sum over the 16 partitions of each e_sub ----
            agg_ps = psum_pool.tile([EG, 4], F32, tag="agg")
            nc.tensor.matmul(agg_ps[:], r_sb[:], stats2[:],
                             start=True, stop=True)
            agg4 = small_pool.tile([EG, 4], F32, tag="agg4_sb")
            nc.vector.tensor_copy(agg4[:], agg_ps[:])
            agg = small_pool.tile([EG, 2], F32, tag="agg_sb")
            nc.vector.tensor_add(agg[:, 0:1], agg4[:, 0:1], agg4[:, 1:2])
            nc.vector.tensor_add(agg[:, 1:2], agg4[:, 2:3], agg4[:, 3:4])

            # ---- tiny per-e math on partitions 0..EG-1 ----
            wv = coef_sb[:, 0 * NG + g:0 * NG + g + 1]     # w_eff
            wsq = coef_sb[:, 1 * NG + g:1 * NG + g + 1]    # w_eff^2
            gam = coef_sb[:, 2 * NG + g:2 * NG + g + 1]    # gamma
            bet = coef_sb[:, 3 * NG + g:3 * NG + g + 1]    # beta

            sm = small_pool.tile([EG, 8], F32, tag="sm")
            mean = sm[:, 0:1]
            mw = sm[:, 1:2]
            ssn = sm[:, 2:3]
            var = sm[:, 3:4]
            sd = sm[:, 4:5]
            rstd = sm[:, 5:6]
            rg = sm[:, 6:7]
            tmp = sm[:, 7:8]
            ab = small_pool.tile([EG, 2], F32, tag="ab")

            # mean = S/N ; mw = w*mean
            nc.vector.tensor_scalar(out=mean, in0=agg[:, 0:1],
                                    scalar1=inv_n, scalar2=None,
                                    op0=mybir.AluOpType.mult)
            nc.vector.tensor_tensor(out=mw, in0=mean, in1=wv,
                                    op=mybir.AluOpType.mult)
            # var' = w^2*SS/N - mw^2
            nc.vector.tensor_scalar(out=ssn, in0=agg[:, 1:2],
                                    scalar1=inv_n, scalar2=None,
                                    op0=mybir.AluOpType.mult)
            nc.vector.tensor_tensor(out=ssn, in0=ssn, in1=wsq,
                                    op=mybir.AluOpType.mult)
            nc.vector.tensor_tensor(out=tmp, in0=mw, in1=mw,
                                    op=mybir.AluOpType.mult)
            nc.vector.tensor_tensor(out=var, in0=ssn, in1=tmp,
                                    op=mybir.AluOpType.subtract)
            # rstd = 1/sqrt(var + eps)   (Rsqrt ACT is banned: sqrt + recip)
            nc.scalar.activation(out=sd, in_=var,
                                 func=mybir.ActivationFunctionType.Sqrt,
                                 bias=eps_t[:])
            nc.vector.reciprocal(rstd, sd)
            # A = w*gamma*rstd ; Bc = beta - mw*gamma*rstd
            nc.vector.tensor_tensor(out=rg, in0=rstd, in1=gam,
                                    op=mybir.AluOpType.mult)
            nc.vector.tensor_tensor(out=ab[:, 0:1], in0=rg, in1=wv,
                                    op=mybir.AluOpType.mult)
            nc.vector.tensor_tensor(out=tmp, in0=mw, in1=rg,
                                    op=mybir.AluOpType.mult)
            nc.vector.tensor_tensor(out=ab[:, 1:2], in0=bet, in1=tmp,
                                    op=mybir.AluOpType.subtract)

            # ---- broadcast A,Bc to per-partition vectors [128, 2] ----
            bc_ps = psum_pool.tile([128, 2], F32, tag="bc")
            nc.tensor.matmul(bc_ps[:], rt_sb[:], ab[:],
                             start=True, stop=True)
            ab_vec = vec_pool.tile([128, 2], F32, tag="abv")
            nc.vector.tensor_copy(ab_vec[:], bc_ps[:])

            # ---- normalize: out = prod*A + Bc ----
            out_t = out_pool.tile([128, L], gdt, tag="outt")
            for h in range(2):
                cs = slice(h * LH, (h + 1) * LH)
                nc.vector.tensor_scalar(out=out_t[:, cs], in0=prod[:, cs],
                                        scalar1=ab_vec[:, 0:1],
                                        scalar2=ab_vec[:, 1:2],
                                        op0=mybir.AluOpType.mult,
                                        op1=mybir.AluOpType.add)
            nc.scalar.dma_start(out_r[g], out_t[:])

    nc.compile()
    return nc


def _get_program(gdt_name=None):
    gdt_name = gdt_name or GATHER_DTYPE
    if gdt_name not in _PROGRAMS:
        _PROGRAMS[gdt_name] = _build_program(gdt_name)
    return _PROGRAMS[gdt_name]


def _host_prep(x, logits, gumbel, tau, gamma, beta):
    """Compute mask indices/weights and build per-core inputs."""
    x = np.asarray(x, dtype=np.float32)
    logits = np.asarray(logits, dtype=np.float32)
    gumbel = np.asarray(gumbel, dtype=np.float32)
    tau_f = np.float32(np.asarray(tau))
    gamma = np.asarray(gamma, dtype=np.float32)
    beta = np.asarray(beta, dtype=np.float32)

    # replicate reference softmax/argmax in fp32 (argmax of z == argmax of
    # softmax(z); verified min top-2 gap 3.4e-4 for these inputs)
    z = (logits + gumbel) / tau_f                     # [2, CE, C1] fp32
    idx = z.argmax(axis=-1)                           # [2, CE]
    zm = z.max(axis=-1, keepdims=True)
    ez = np.exp(z - zm, dtype=np.float32)
    soft = ez / ez.sum(axis=-1, keepdims=True, dtype=np.float32)
    s_hot = np.take_along_axis(soft, idx[..., None], axis=-1)[..., 0]
    w = (np.float32(1.0) - s_hot) + s_hot             # [2, CE] (== 1.0 here)
    weff = (w[0] * w[1]).astype(np.float32)           # [CE]

    # channel-major copy of x for fast row gathers: [C1, B*L]
    xt = np.ascontiguousarray(
        x.reshape(B, C1, L).transpose(1, 0, 2)).reshape(C1, N)
    if GATHER_DTYPE == "f16":
        xt = xt.astype(np.float16)

    # R / R^T block one-hot (partition p belongs to e_sub = p//B)
    rmat = np.zeros((128, EG), dtype=np.float32)
    for es in range(EG):
        rmat[es * B:(es + 1) * B, es] = 1.0
    rtmat = np.ascontiguousarray(rmat.T)

    in_maps = []
    for k in range(NCORES):
        e0 = k * EPC
        rows = np.concatenate([idx[0, e0:e0 + EPC], idx[1, e0:e0 + EPC]])
        xsel = np.ascontiguousarray(xt[rows])         # [128, N]

        coef = np.zeros((EG, 4 * NG), dtype=np.float32)
        for g in range(NG):
            el = e0 + g * EG + np.arange(EG)          # global e for (g, e_sub)
            coef[:, 0 * NG + g] = weff[el]
            coef[:, 1 * NG + g] = weff[el] * weff[el]
            coef[:, 2 * NG + g] = gamma[el]
            coef[:, 3 * NG + g] = beta[el]

        in_maps.append({
            "xsel": xsel,
            "coef": coef,
            "rmat": rmat,
            "rtmat": rtmat,
        })
    return in_maps


def _install_ntff_shim():
    """The agent image's antenv lacks axon_hooks; recreate it so
    run_bass_kernel_spmd(trace=True) can capture NTFF profiles."""
    import types
    if "antenv.axon_hooks" in sys.modules:
        return
    mod = types.ModuleType("antenv.axon_hooks")
    _hook = [None]
    mod.set_axon_ntff_profile_hook = lambda h: _hook.__setitem__(0, h)
    mod.get_axon_ntff_profile_hook = lambda: _hook[0]
    sys.modules["antenv.axon_hooks"] = mod
    import antenv
    antenv.axon_hooks = mod
    from trn_agent_boot.trn_boot import _ntff_profile_via_ctypes
    mod.set_axon_ntff_profile_hook(
        _ntff_profile_via_ctypes("/opt/axon/libaxon_pjrt.so"))


def kernel(x, logits, gumbel, tau, gamma, beta):
    global LAST_RESULT
    nc = _get_program()
    in_maps = _host_prep(x, logits, gumbel, tau, gamma, beta)

    trace = bool(int(os.environ.get("KERNEL_PROFILE", "0")))
    if trace:
        try:
            _install_ntff_shim()
        except Exception:
            trace = False
    try:
        res = run_bass_kernel_spmd(nc, in_maps, list(range(NCORES)),
                                   trace=trace)
    except Exception:
        if not trace:
            raise
        res = run_bass_kernel_spmd(nc, in_maps, list(range(NCORES)),
                                   trace=False)
    LAST_RESULT = res

    out = np.empty((B, CE, L), dtype=np.float32)
    for k in range(NCORES):
        out[:, k * EPC:(k + 1) * EPC, :] = res.results[k]["out"].transpose(1, 0, 2)
    return out.reshape(B, CE, H, W)

